# revision 14
# baseline (speedup 1.0000x reference)
"""Trainium2 Bass kernel for nn_Actions_Emb (ragged char-mean + action embedding).

v6.2 design: type-compacted slots (host-side permutation).

The three slot types are mutually exclusive per slot, so the host
partitions each core's 8192 slots by type and the device only computes
content rows:
  - type-0 (char-mean) slots, padded to 3072 (24 tiles of 128):
      counts[slot, c] = #{l < len : char_ids[slot, l] == c} via 58
      DVE compares on sentinel-masked bf16 ids (4x mode), summed over l
      on PE (16 accumulating identity matmuls per class chunk; the PE
      p-state is fully ramped after ~3us so these run at full rate),
      scaled by 1/len on DVE, PE-transposed per tile to class-major,
      one K=58 matmul per tile against the char table.
  - type-1 (action) slots, padded to 3072: action one-hot built
      class-major (ids staged to DRAM, DMA-broadcast across 99
      partitions, one Pool compare vs the partition-index column), one
      K=99 matmul per tile against the action table.
  - type-2 rows are zero and BOS rows are a broadcast of one table row;
      the host fills those during output assembly (buffer init), and
      scatters the device rows back to their original positions.

PSUM quad evacuations are spread across Act/Pool (action section, under
the compare phase) and DVE/Act (char tail); outputs leave in 8-tile
DMAs. Output is bf16 (host widens); rel err ~4e-3 << 2e-2 gate.
"""

import numpy as np
import sys

if "/opt/trn_rl_repo" not in sys.path:
    sys.path.insert(0, "/opt/trn_rl_repo")

import concourse.bass as bass
import concourse.bacc as bacc
import concourse.mybir as mybir
import concourse.tile as tile
from concourse.bass import AP
from concourse.bass_utils import run_bass_kernel_spmd
from concourse.masks import make_identity

B, S, L, D = 16384, 4, 16, 256
NCHAR, NACT, BOS_ID = 58, 99, 98
NCORES = 8
B_CORE = B // NCORES           # 2048 proof steps per core
SLOTS = B_CORE * S             # 8192 slots per core
P = 128
NT = 24                        # tiles per compacted section (3072 slots)
SECT = NT * P                  # 3072; covers max per-type count (~2814)
NQ = NT // 4                   # 6 quads per section
CHUNKS = [(0, 8), (8, 16), (24, 16), (40, 16), (56, 2)]   # 58 classes

f32 = mybir.dt.float32
bf16 = mybir.dt.bfloat16
i32 = mybir.dt.int32
Alu = mybir.AluOpType

_CACHE = {}


def build_nc():
    nc = bacc.Bacc("TRN2", target_bir_lowering=False, debug=False,
                   num_devices=NCORES)

    cids_d = nc.dram_tensor("cids", [SECT, L], i32, kind="ExternalInput")
    clen_d = nc.dram_tensor("clen", [SECT], i32, kind="ExternalInput")
    aids_d = nc.dram_tensor("aids", [SECT], i32, kind="ExternalInput")
    ct_d = nc.dram_tensor("char_table", [NCHAR, D], f32, kind="ExternalInput")
    at_d = nc.dram_tensor("action_table", [NACT, D], f32, kind="ExternalInput")
    outc_d = nc.dram_tensor("out_c", [SECT, D], bf16, kind="ExternalOutput")
    outa_d = nc.dram_tensor("out_a", [SECT, D], bf16, kind="ExternalOutput")
    scr_d = nc.dram_tensor("act_scratch", [SECT], bf16, kind="Internal")

    # compacted slot k = p*NT + t
    cids_r = cids_d.rearrange("(p t) l -> p t l", p=P)      # [128, 24, 16]
    clen_r = clen_d.rearrange("(p t) -> p t", p=P)          # [128, 24]
    aids_r = aids_d.rearrange("(p t) -> p t", p=P)
    outc_r = outc_d.rearrange("(p x) d -> p x d", p=P)      # [128, 24, 256]
    outa_r = outa_d.rearrange("(p x) d -> p x d", p=P)
    scr_r = scr_d.rearrange("(p t) -> p t", p=P)
    scr_row = scr_d.rearrange("(a n) -> a n", a=1)          # [1, 3072]

    from contextlib import ExitStack
    with tile.TileContext(nc) as tc, ExitStack() as es:
        consts = es.enter_context(tc.tile_pool(name="consts", bufs=1))
        big = es.enter_context(tc.tile_pool(name="big", bufs=1))
        qpp = es.enter_context(tc.tile_pool(name="qp", bufs=3, space="PSUM"))

        # ---- constants ----
        ident_bf = consts.tile([P, P], bf16)
        make_identity(nc, ident_bf)

        iotacol_i = consts.tile([P, 1], i32)
        nc.gpsimd.iota(iotacol_i, pattern=[[1, 1]], base=0, channel_multiplier=1)
        iotacol = consts.tile([P, 1], f32)
        nc.vector.tensor_copy(iotacol, iotacol_i)
        # iota over l, [P, NT, L], value = l (cast to bf16 on Pool)
        iotal_i = consts.tile([P, NT, L], i32)
        nc.gpsimd.iota(iotal_i, pattern=[[0, NT], [1, L]], base=0,
                       channel_multiplier=0)
        iotal = consts.tile([P, NT, L], bf16)
        nc.gpsimd.tensor_copy(iotal, iotal_i)

        # action ids: one dependency-free casting DMA (i32 -> bf16,
        # DRAM -> DRAM) feeds the class-major broadcast immediately
        aids_row = aids_d.rearrange("(a n) -> a n", a=1)
        nc.gpsimd.dma_start(scr_row, aids_row)

        # ---- bulk input loads (len/ids first: they gate the compare chain) ----
        len_i = big.tile([P, NT], i32)
        nc.sync.dma_start(len_i, clen_r)
        ids_i = big.tile([P, NT, L], i32)
        nc.sync.dma_start(ids_i, cids_r)
        ct32 = consts.tile([NCHAR, D], f32)
        nc.sync.dma_start(ct32, ct_d[:, :])
        at32 = consts.tile([NACT, D], f32)
        nc.sync.dma_start(at32, at_d[:, :])

        lenf = big.tile([P, NT], f32)
        nc.vector.tensor_copy(lenf, len_i)
        ids_bf = big.tile([P, NT, L], bf16)
        nc.vector.tensor_copy(ids_bf, ids_i)
        # mask64[p,t,l] = 64 * (l >= len[p,t]); sentinel keeps masked chars
        # out of every class compare (ids_m <= 121, exact in bf16)
        lenb = lenf[:, :].unsqueeze(2).broadcast_to((P, NT, L))
        m64 = big.tile([P, NT, L], bf16)
        nc.vector.tensor_tensor(out=m64, in0=iotal, in1=lenb, op=Alu.is_ge)
        m64s = big.tile([P, NT, L], bf16)
        nc.vector.tensor_scalar(out=m64s, in0=m64, scalar1=64.0, scalar2=None,
                                op0=Alu.mult)
        ids_m = big.tile([P, NT, L], bf16)
        nc.vector.tensor_tensor(out=ids_m, in0=ids_bf, in1=m64s, op=Alu.add)

        rlen = big.tile([P, NT], f32)
        nc.vector.reciprocal(rlen, lenf)

        act_rep = big.tile([NACT, SECT], bf16)
        src1 = scr_row[0:1, :]
        rep_ap = src1.ap.copy()
        rep_ap[0] = (0, NACT)                     # partition stride 0
        rep_src = AP(src1.tensor, src1.offset, rep_ap)
        nc.sync.dma_start(act_rep, rep_src)

        # one-hot compare on Pool in slices (DVE is the bottleneck engine;
        # slices let the first action matmuls start earlier)
        wa_t = big.tile([NACT, SECT], bf16)
        for g in range(6):
            sl = slice(g * SECT // 6, (g + 1) * SECT // 6)
            nc.gpsimd.tensor_scalar(out=wa_t[:, sl], in0=act_rep[:, sl],
                                    scalar1=iotacol[:NACT, 0:1], scalar2=None,
                                    op0=Alu.is_equal)
        wa_v = wa_t[:, :].rearrange("c (p t) -> c t p", t=NT)

        # table casts (off the compare-critical head)
        ct_sb = consts.tile([NCHAR, D], bf16)
        nc.vector.tensor_copy(ct_sb, ct32)
        at_sb = consts.tile([NACT, D], bf16)
        nc.vector.tensor_copy(at_sb, at32)

        obuf_a = big.tile([P, NT, D], bf16)
        obuf_c = big.tile([P, NT, D], bf16)

        # warmup source for the PE p-state ramp (matmuls emitted below,
        # sharing the count-chunk PSUM pool)
        wmsrc = consts.tile([P, D], bf16)
        nc.gpsimd.memset(wmsrc, 0.0)

        def quad(q, mm_one, obuf, out_r, evac_one):
            out_p = qpp.tile([P, 4, D], f32, tag="qp", name=f"qp_{obuf.name}{q}")
            for j in range(4):
                mm_one(out_p, 4 * q + j, j)
            evac_one(q, obuf, out_p)
            if q % 2 == 1:
                nc.sync.dma_start(out_r[:, 8 * (q // 2):8 * (q // 2) + 8, :],
                                  obuf[:, 8 * (q // 2):8 * (q // 2) + 8, :])

        def act_evac(q, obuf, out_p):
            nc.scalar.copy(obuf[:, 4 * q:4 * q + 4, :], out_p)

        def char_evac(q, obuf, out_p):
            # per-tile evac folds the 1/len scale (reference divides after
            # the masked char-sum); DVE and Act alternate
            for j in range(4):
                t = 4 * q + j
                if t % 3 != 2:
                    nc.vector.tensor_scalar(out=obuf[:, t, :],
                                            in0=out_p[:, j, :],
                                            scalar1=rlen[:, t:t + 1],
                                            scalar2=None, op0=Alu.mult)
                else:
                    nc.scalar.mul(obuf[:, t, :], out_p[:, j, :],
                                  rlen[:, t:t + 1])

        def act_mm(out_p, t, j):
            nc.tensor.matmul(out_p[:, j, :], wa_v[:, t, :], at_sb,
                             start=True, stop=True)

        def char_mm(out_p, t, j):
            nc.tensor.matmul(out_p[:, j, :], wct[:NCHAR, t, :], ct_sb,
                             start=True, stop=True)

        # ---- per-class counts + action quads interleaved on PE ----
        counts = big.tile([P, NT, 64], bf16)
        nc.gpsimd.memset(counts[:, :, NCHAR:64], 0.0)   # pad classes


        with (
            tc.tile_pool(name="eq", bufs=5) as eqp,
            tc.tile_pool(name="cc", bufs=2, space="PSUM") as ccp,
        ):
            wm = ccp.tile([P, D], f32, tag="ps", name="wm")
            for _ in range(8):
                nc.tensor.matmul(wm, ident_bf, wmsrc, start=True, stop=True)
            for ci, (c0, csz) in enumerate(CHUNKS):
                eq = eqp.tile([P, NT, csz, L], bf16, tag="eq", name=f"eq{ci}")
                for c in range(csz):
                    nc.vector.tensor_scalar(out=eq[:, :, c, :], in0=ids_m,
                                            scalar1=float(c0 + c),
                                            scalar2=None, op0=Alu.is_equal)
                ps = ccp.tile([P, NT, csz], f32, tag="ps", name=f"ps{ci}")
                for l in range(L):
                    nc.tensor.matmul(ps, ident_bf, eq[:, :, :, l],
                                     start=(l == 0), stop=(l == L - 1))
                nc.scalar.copy(counts[:, :, c0:c0 + csz], ps)
                # action quads slot between count chunks on PE
                if ci < NQ - 1:
                    quad(ci, act_mm, obuf_a, outa_r, act_evac)
            for q in range(len(CHUNKS) - 1, NQ):
                quad(q, act_mm, obuf_a, outa_r, act_evac)

        # ---- batched scale, transpose, char gather, emit ----
        with tc.tile_pool(name="wp", bufs=1, space="PSUM") as wpp:
            wct = big.tile([64, NT, P], bf16)
            for h in range(2):      # two 12-tile halves through a 2-bank pool
                t0 = 12 * h
                wct_p = wpp.tile([64, 12, P], bf16, tag="wctp", name=f"wp{h}")
                for t in range(12):
                    nc.tensor.transpose(wct_p[:, t, :], counts[:, t0 + t, :],
                                        ident_bf)
                # PSUM->SBUF evacs, DVE/Act split
                nc.vector.tensor_copy(wct[:, t0:t0 + 6, :], wct_p[:, 0:6, :])
                nc.scalar.copy(wct[:, t0 + 6:t0 + 12, :], wct_p[:, 6:12, :])
            for q in range(NQ):
                quad(q, char_mm, obuf_c, outc_r, char_evac)
    nc.compile()
    return nc


def kernel(**inputs):
    char_ids = np.ascontiguousarray(np.asarray(inputs["char_ids"], np.int32))
    char_len = np.ascontiguousarray(np.asarray(inputs["char_len"], np.int32))
    action_ids = np.ascontiguousarray(np.asarray(inputs["action_ids"], np.int32))
    slot_type = np.ascontiguousarray(np.asarray(inputs["slot_type"], np.int32))
    char_table = np.ascontiguousarray(np.asarray(inputs["char_table"], np.float32))
    action_table = np.ascontiguousarray(np.asarray(inputs["action_table"], np.float32))

    ids_f = char_ids.reshape(B * S, L)
    len_f = char_len.reshape(B * S)
    act_f = action_ids.reshape(B * S)
    typ_f = slot_type.reshape(B * S)

    if "nc" not in _CACHE:
        _CACHE["nc"] = build_nc()
    nc = _CACHE["nc"]

    in_maps = []
    idx0s, idx1s = [], []
    for c in range(NCORES):
        lo = c * SLOTS
        seg = typ_f[lo:lo + SLOTS]
        idx0 = np.flatnonzero(seg == 0)
        idx1 = np.flatnonzero(seg == 1)
        if len(idx0) > SECT or len(idx1) > SECT:
            raise RuntimeError(
                f"type-compacted section overflow: {len(idx0)}/{len(idx1)} > {SECT}")
        idx0s.append(idx0)
        idx1s.append(idx1)

        cids = np.zeros((SECT, L), np.int32)
        cids[:len(idx0)] = ids_f[lo + idx0]
        clen = np.ones(SECT, np.int32)
        clen[:len(idx0)] = len_f[lo + idx0]
        aids = np.zeros(SECT, np.int32)
        aids[:len(idx1)] = act_f[lo + idx1]

        in_maps.append({
            "cids": cids,
            "clen": clen,
            "aids": aids,
            "char_table": char_table,
            "action_table": action_table,
        })

    res = run_bass_kernel_spmd(nc, in_maps, list(range(NCORES)))
    _CACHE["last_res"] = res

    out = np.zeros((B, 5, D), np.float32)
    out[:, 0, :] = action_table[BOS_ID]
    flat = out.reshape(B * 5, D)
    for c in range(NCORES):
        lo = c * SLOTS
        outc = np.asarray(res.results[c]["out_c"]).astype(np.float32)
        outa = np.asarray(res.results[c]["out_a"]).astype(np.float32)
        g0 = lo + idx0s[c]                 # global slot index
        g1 = lo + idx1s[c]
        flat[(g0 // S) * 5 + 1 + g0 % S] = outc[:len(idx0s[c])]
        flat[(g1 // S) * 5 + 1 + g1 % S] = outa[:len(idx1s[c])]
    return out


if __name__ == "__main__":
    import reference
    inp = {k: np.asarray(v) for k, v in reference.setup_inputs().items()}
    got = kernel(**inp)
    exp = np.asarray(reference.reference(**inp))
    err = np.abs(got - exp).max() / (np.abs(exp).max() + 1e-9)
    print("rel err:", err)


# revision 15
# speedup vs baseline: 1.1035x; 1.1035x over previous
"""Trainium2 Bass kernel for nn_Actions_Emb (ragged char-mean + action embedding).

v6.2 design: type-compacted slots (host-side permutation).

The three slot types are mutually exclusive per slot, so the host
partitions each core's 8192 slots by type and the device only computes
content rows:
  - type-0 (char-mean) slots, padded to 3072 (24 tiles of 128):
      counts[slot, c] = #{l < len : char_ids[slot, l] == c} via 58
      DVE compares on sentinel-masked bf16 ids (4x mode), summed over l
      on PE (16 accumulating identity matmuls per class chunk; the PE
      p-state is fully ramped after ~3us so these run at full rate),
      scaled by 1/len on DVE, PE-transposed per tile to class-major,
      one K=58 matmul per tile against the char table.
  - type-1 (action) slots, padded to 3072: action one-hot built
      class-major (ids staged to DRAM, DMA-broadcast across 99
      partitions, one Pool compare vs the partition-index column), one
      K=99 matmul per tile against the action table.
  - type-2 rows are zero and BOS rows are a broadcast of one table row;
      the host fills those during output assembly (buffer init), and
      scatters the device rows back to their original positions.

PSUM quad evacuations are spread across Act/Pool (action section, under
the compare phase) and DVE/Act (char tail); outputs leave in 8-tile
DMAs. Output is bf16 (host widens); rel err ~4e-3 << 2e-2 gate.
"""

import numpy as np
import sys

if "/opt/trn_rl_repo" not in sys.path:
    sys.path.insert(0, "/opt/trn_rl_repo")

import concourse.bass as bass
import concourse.bacc as bacc
import concourse.mybir as mybir
import concourse.tile as tile
from concourse.bass import AP
from concourse.bass_utils import run_bass_kernel_spmd
from concourse.masks import make_identity

B, S, L, D = 16384, 4, 16, 256
NCHAR, NACT, BOS_ID = 58, 99, 98
NCORES = 8
B_CORE = B // NCORES           # 2048 proof steps per core
SLOTS = B_CORE * S             # 8192 slots per core
P = 128
NT = 24                        # tiles per compacted section (3072 slots)
SECT = NT * P                  # 3072; covers max per-type count (~2814)
NQ = NT // 4                   # 6 quads per section
CHUNKS = [(0, 8), (8, 16), (24, 16), (40, 16), (56, 2)]   # 58 classes

f32 = mybir.dt.float32
bf16 = mybir.dt.bfloat16
i32 = mybir.dt.int32
Alu = mybir.AluOpType

_CACHE = {}


def build_nc():
    nc = bacc.Bacc("TRN2", target_bir_lowering=False, debug=False,
                   num_devices=NCORES)

    cids_d = nc.dram_tensor("cids", [SECT, L], i32, kind="ExternalInput")
    clen_d = nc.dram_tensor("clen", [SECT], i32, kind="ExternalInput")
    aids_d = nc.dram_tensor("aids", [SECT], i32, kind="ExternalInput")
    ct_d = nc.dram_tensor("char_table", [NCHAR, D], f32, kind="ExternalInput")
    at_d = nc.dram_tensor("action_table", [NACT, D], f32, kind="ExternalInput")
    outc_d = nc.dram_tensor("out_c", [SECT, D], bf16, kind="ExternalOutput")
    outa_d = nc.dram_tensor("out_a", [SECT, D], bf16, kind="ExternalOutput")
    scr_d = nc.dram_tensor("act_scratch", [SECT], bf16, kind="Internal")

    # compacted slot k = p*NT + t
    cids_r = cids_d.rearrange("(p t) l -> p t l", p=P)      # [128, 24, 16]
    clen_r = clen_d.rearrange("(p t) -> p t", p=P)          # [128, 24]
    aids_r = aids_d.rearrange("(p t) -> p t", p=P)
    outc_r = outc_d.rearrange("(x p) d -> p x d", p=P)      # row = t*128+p
    outa_r = outa_d.rearrange("(x p) d -> p x d", p=P)
    scr_r = scr_d.rearrange("(p t) -> p t", p=P)
    scr_row = scr_d.rearrange("(a n) -> a n", a=1)          # [1, 3072]

    from contextlib import ExitStack
    with tile.TileContext(nc) as tc, ExitStack() as es:
        consts = es.enter_context(tc.tile_pool(name="consts", bufs=1))
        big = es.enter_context(tc.tile_pool(name="big", bufs=1))
        qpp = es.enter_context(tc.tile_pool(name="qp", bufs=3, space="PSUM"))

        # ---- constants ----
        ident_bf = consts.tile([P, P], bf16)
        make_identity(nc, ident_bf)

        # action ids: dependency-free casting DMA (i32 -> bf16, DRAM->DRAM)
        aids_row = aids_d.rearrange("(a n) -> a n", a=1)
        nc.gpsimd.dma_start(scr_row, aids_row)

        iotacol_i = consts.tile([P, 1], i32)
        nc.gpsimd.iota(iotacol_i, pattern=[[1, 1]], base=0, channel_multiplier=1)
        iotacol = consts.tile([P, 1], f32)
        nc.vector.tensor_copy(iotacol, iotacol_i)
        # iota over l, [P, NT, L], value = l (cast to bf16 on Pool)
        iotal_i = consts.tile([P, NT, L], i32)
        nc.gpsimd.iota(iotal_i, pattern=[[0, NT], [1, L]], base=0,
                       channel_multiplier=0)
        iotal = consts.tile([P, NT, L], bf16)
        nc.gpsimd.tensor_copy(iotal, iotal_i)

        # ---- bulk input loads (len/ids first: they gate the compare chain) ----
        len_i = big.tile([P, NT], i32)
        nc.sync.dma_start(len_i, clen_r)
        ids_i = big.tile([P, NT, L], i32)
        nc.sync.dma_start(ids_i, cids_r)
        ct32 = consts.tile([NCHAR, D], f32)
        nc.sync.dma_start(ct32, ct_d[:, :])
        at32 = consts.tile([NACT, D], f32)
        nc.sync.dma_start(at32, at_d[:, :])

        lenf = big.tile([P, NT], f32)
        nc.vector.tensor_copy(lenf, len_i)
        ids_bf = big.tile([P, NT, L], bf16)
        nc.vector.tensor_copy(ids_bf, ids_i)
        # mask64[p,t,l] = 64 * (l >= len[p,t]); sentinel keeps masked chars
        # out of every class compare (ids_m <= 121, exact in bf16)
        lenb = lenf[:, :].unsqueeze(2).broadcast_to((P, NT, L))
        m64 = big.tile([P, NT, L], bf16)
        nc.vector.tensor_tensor(out=m64, in0=iotal, in1=lenb, op=Alu.is_ge)
        m64s = big.tile([P, NT, L], bf16)
        nc.vector.tensor_scalar(out=m64s, in0=m64, scalar1=64.0, scalar2=None,
                                op0=Alu.mult)
        ids_m = big.tile([P, NT, L], bf16)
        nc.vector.tensor_tensor(out=ids_m, in0=ids_bf, in1=m64s, op=Alu.add)

        rlen = big.tile([P, NT], f32)
        nc.vector.reciprocal(rlen, lenf)

        act_rep = big.tile([NACT, SECT], bf16)
        src1 = scr_row[0:1, :]
        rep_ap = src1.ap.copy()
        rep_ap[0] = (0, NACT)                     # partition stride 0
        rep_src = AP(src1.tensor, src1.offset, rep_ap)
        nc.sync.dma_start(act_rep, rep_src)

        # one-hot compare on Pool in slices (DVE is the bottleneck engine;
        # slices let the first action matmuls start earlier)
        wa_t = big.tile([NACT, SECT], bf16)
        for g in range(NQ):          # slice g = quad g's 512 slots (t-major)
            sl = slice(g * 4 * P, (g + 1) * 4 * P)
            nc.gpsimd.tensor_scalar(out=wa_t[:, sl], in0=act_rep[:, sl],
                                    scalar1=iotacol[:NACT, 0:1], scalar2=None,
                                    op0=Alu.is_equal)

        # table casts (off the compare-critical head)
        ct_sb = consts.tile([NCHAR, D], bf16)
        nc.vector.tensor_copy(ct_sb, ct32)
        at_sb = consts.tile([NACT, D], bf16)
        nc.vector.tensor_copy(at_sb, at32)

        obuf_a = big.tile([P, NT, D], bf16)
        obuf_c = big.tile([P, NT, D], bf16)

        # warmup source for the PE p-state ramp (matmuls emitted below,
        # sharing the count-chunk PSUM pool)
        wmsrc = consts.tile([P, D], bf16)
        nc.gpsimd.memset(wmsrc, 0.0)

        def quad(q, mm_one, obuf, out_r, evac_one, dma_quads=2):
            out_p = qpp.tile([P, 4, D], f32, tag="qp", name=f"qp_{obuf.name}{q}")
            for j in range(4):
                mm_one(out_p, 4 * q + j, j)
            evac_one(q, obuf, out_p)
            if q % dma_quads == dma_quads - 1:
                lo = 4 * (q - dma_quads + 1)
                nc.sync.dma_start(out_r[:, lo:4 * q + 4, :],
                                  obuf[:, lo:4 * q + 4, :])

        def act_evac(q, obuf, out_p):
            nc.scalar.copy(obuf[:, 4 * q:4 * q + 4, :], out_p)

        def char_evac(q, obuf, out_p):
            # per-tile evac folds the 1/len scale (reference divides after
            # the masked char-sum); DVE and Act alternate
            for j in range(4):
                t = 4 * q + j
                if t % 3 != 2:
                    nc.vector.tensor_scalar(out=obuf[:, t, :],
                                            in0=out_p[:, j, :],
                                            scalar1=rlen[:, t:t + 1],
                                            scalar2=None, op0=Alu.mult)
                else:
                    nc.scalar.mul(obuf[:, t, :], out_p[:, j, :],
                                  rlen[:, t:t + 1])

        def act_mm(out_p, t, j):
            nc.tensor.matmul(out_p[:, j, :], wa_t[:, P * t:P * (t + 1)], at_sb,
                             start=True, stop=True)

        def char_mm(out_p, t, j):
            nc.tensor.matmul(out_p[:, j, :], wct[:NCHAR, t, :], ct_sb,
                             start=True, stop=True)

        # ---- per-class counts + action quads interleaved on PE ----
        counts = big.tile([P, NT, 64], bf16)
        nc.gpsimd.memset(counts[:, :, NCHAR:64], 0.0)   # pad classes


        with (
            tc.tile_pool(name="eq", bufs=5) as eqp,
            tc.tile_pool(name="cc", bufs=2, space="PSUM") as ccp,
        ):
            wm = ccp.tile([P, D], f32, tag="ps", name="wm")
            for _ in range(8):
                nc.tensor.matmul(wm, ident_bf, wmsrc, start=True, stop=True)
            for ci, (c0, csz) in enumerate(CHUNKS):
                eq = eqp.tile([P, NT, csz, L], bf16, tag="eq", name=f"eq{ci}")
                for c in range(csz):
                    nc.vector.tensor_scalar(out=eq[:, :, c, :], in0=ids_m,
                                            scalar1=float(c0 + c),
                                            scalar2=None, op0=Alu.is_equal)
                ps = ccp.tile([P, NT, csz], f32, tag="ps", name=f"ps{ci}")
                for l in range(L):
                    nc.tensor.matmul(ps, ident_bf, eq[:, :, :, l],
                                     start=(l == 0), stop=(l == L - 1))
                nc.scalar.copy(counts[:, :, c0:c0 + csz], ps)
                # action quads slot between count chunks on PE
                if ci < NQ - 1:
                    quad(ci, act_mm, obuf_a, outa_r, act_evac)
            for q in range(len(CHUNKS) - 1, NQ):
                quad(q, act_mm, obuf_a, outa_r, act_evac)

        # ---- batched scale, transpose, char gather, emit ----
        with tc.tile_pool(name="wp", bufs=1, space="PSUM") as wpp:
            wct = big.tile([64, NT, P], bf16)
            for h in range(2):      # two 12-tile halves through a 2-bank pool
                t0 = 12 * h
                wct_p = wpp.tile([64, 12, P], bf16, tag="wctp", name=f"wp{h}")
                for t in range(12):
                    nc.tensor.transpose(wct_p[:, t, :], counts[:, t0 + t, :],
                                        ident_bf)
                # PSUM->SBUF evacs, DVE/Act split
                nc.vector.tensor_copy(wct[:, t0:t0 + 6, :], wct_p[:, 0:6, :])
                nc.scalar.copy(wct[:, t0 + 6:t0 + 12, :], wct_p[:, 6:12, :])
            for q in range(NQ):
                quad(q, char_mm, obuf_c, outc_r, char_evac, dma_quads=1)
    nc.compile()
    return nc


def kernel(**inputs):
    char_ids = np.ascontiguousarray(np.asarray(inputs["char_ids"], np.int32))
    char_len = np.ascontiguousarray(np.asarray(inputs["char_len"], np.int32))
    action_ids = np.ascontiguousarray(np.asarray(inputs["action_ids"], np.int32))
    slot_type = np.ascontiguousarray(np.asarray(inputs["slot_type"], np.int32))
    char_table = np.ascontiguousarray(np.asarray(inputs["char_table"], np.float32))
    action_table = np.ascontiguousarray(np.asarray(inputs["action_table"], np.float32))

    ids_f = char_ids.reshape(B * S, L)
    len_f = char_len.reshape(B * S)
    act_f = action_ids.reshape(B * S)
    typ_f = slot_type.reshape(B * S)

    if "nc" not in _CACHE:
        _CACHE["nc"] = build_nc()
    nc = _CACHE["nc"]

    in_maps = []
    idx0s, idx1s = [], []
    for c in range(NCORES):
        lo = c * SLOTS
        seg = typ_f[lo:lo + SLOTS]
        idx0 = np.flatnonzero(seg == 0)
        idx1 = np.flatnonzero(seg == 1)
        if len(idx0) > SECT or len(idx1) > SECT:
            raise RuntimeError(
                f"type-compacted section overflow: {len(idx0)}/{len(idx1)} > {SECT}")
        idx0s.append(idx0)
        idx1s.append(idx1)

        # compacted slot k = t*128 + p (t-major); cids/clen are laid out
        # p-major on DRAM so their DMAs stay contiguous per partition
        cids = np.zeros((SECT, L), np.int32)
        cids[:len(idx0)] = ids_f[lo + idx0]
        cids = np.ascontiguousarray(
            cids.reshape(NT, P, L).transpose(1, 0, 2).reshape(SECT, L))
        clen = np.ones(SECT, np.int32)
        clen[:len(idx0)] = len_f[lo + idx0]
        clen = np.ascontiguousarray(clen.reshape(NT, P).T.reshape(SECT))
        aids = np.zeros(SECT, np.int32)
        aids[:len(idx1)] = act_f[lo + idx1]

        in_maps.append({
            "cids": cids,
            "clen": clen,
            "aids": aids,
            "char_table": char_table,
            "action_table": action_table,
        })

    res = run_bass_kernel_spmd(nc, in_maps, list(range(NCORES)))
    _CACHE["last_res"] = res

    out = np.zeros((B, 5, D), np.float32)
    out[:, 0, :] = action_table[BOS_ID]
    flat = out.reshape(B * 5, D)
    for c in range(NCORES):
        lo = c * SLOTS
        outc = np.asarray(res.results[c]["out_c"]).astype(np.float32)
        outa = np.asarray(res.results[c]["out_a"]).astype(np.float32)
        g0 = lo + idx0s[c]                 # global slot index
        g1 = lo + idx1s[c]
        flat[(g0 // S) * 5 + 1 + g0 % S] = outc[:len(idx0s[c])]
        flat[(g1 // S) * 5 + 1 + g1 % S] = outa[:len(idx1s[c])]
    return out


if __name__ == "__main__":
    import reference
    inp = {k: np.asarray(v) for k, v in reference.setup_inputs().items()}
    got = kernel(**inp)
    exp = np.asarray(reference.reference(**inp))
    err = np.abs(got - exp).max() / (np.abs(exp).max() + 1e-9)
    print("rel err:", err)
